# revision 1
# baseline (speedup 1.0000x reference)
import sys
sys.path.insert(0, "/opt/trn_rl_repo")
import numpy as np
import concourse.bass as bass
import concourse.mybir as mybir
import concourse.tile as tile
from concourse import bacc
from concourse.bass_utils import run_bass_kernel_spmd
from concourse.masks import make_identity

F32 = mybir.dt.float32
F32R = mybir.dt.float32r
AF = mybir.ActivationFunctionType
OP = mybir.AluOpType

S = 2048          # sequence length
H = 4096          # hidden dim
DHEAD = 128       # head dim
NQ = 4            # q heads per core
NCORES = 8
SC = 4            # s-chunks of 512
HO = 32           # h k-tiles of 128
SCALE = 1.0 / np.sqrt(128.0)

_CACHED = {}


def _build_nc():
    nc = bacc.Bacc(None, target_bir_lowering=False, debug=False)
    hid_d = nc.dram_tensor("hidden", [S, H], F32, kind="ExternalInput")
    wqkv_d = nc.dram_tensor("wqkv", [768, H], F32, kind="ExternalInput")
    wo_d = nc.dram_tensor("wo", [H, 512], F32, kind="ExternalInput")
    cos_d = nc.dram_tensor("cos", [128, S], F32, kind="ExternalInput")
    sin_d = nc.dram_tensor("sin", [128, S], F32, kind="ExternalInput")
    out_d = nc.dram_tensor("out", [S, H], F32, kind="ExternalOutput")

    with tile.TileContext(nc) as tc:
        with tc.tile_pool(name="perm", bufs=1) as perm:
            ident = perm.tile([128, 128], F32, tag="ident")
            make_identity(nc, ident)
            identr = perm.tile([128, 128], F32R, tag="identr")
            nc.vector.tensor_copy(identr, ident)
            ones_f = perm.tile([128, 128], F32, tag="ones_f")
            nc.gpsimd.memset(ones_f, 1.0)
            ones = perm.tile([128, 128], F32R, tag="ones")
            nc.vector.tensor_copy(ones, ones_f)
            # qT/kT/vT strips, [d=128, strip, s]: strips 0-3 = Q heads, 4 = K, 5 = V
            strips = perm.tile([128, 6, S], F32R, tag="strips")

            # ---------------- Phase B: projections ----------------
            with tc.tile_pool(name="wtp", bufs=1) as wtp, \
                 tc.tile_pool(name="wn", bufs=2) as wn_p, \
                 tc.tile_pool(name="hp", bufs=2) as hp_p, \
                 tc.tile_pool(name="ht", bufs=3) as ht_p, \
                 tc.tile_pool(name="cs", bufs=2) as cs_p, \
                 tc.tile_pool(name="rt", bufs=2) as rt_p, \
                 tc.tile_pool(name="ppj", bufs=1, space="PSUM") as ppj, \
                 tc.tile_pool(name="ptr", bufs=2, space="PSUM") as ptr:

                # transpose W_qkv [768, H] -> wt [128, HO, 768] (f32r)
                wt = wtp.tile([128, HO, 768], F32R, tag="wt")
                for dt in range(6):
                    for part in range(8):
                        wn = wn_p.tile([128, 512], F32, tag="wn")
                        nc.sync.dma_start(
                            wn, wqkv_d[dt * 128:(dt + 1) * 128, part * 512:(part + 1) * 512])
                        pt4 = ptr.tile([128, 512], F32, tag="tp")
                        for j in range(4):
                            nc.tensor.transpose(
                                pt4[:, j * 128:(j + 1) * 128],
                                wn[:, j * 128:(j + 1) * 128], ident)
                        nc.scalar.copy(
                            wt[:, part * 4:(part + 1) * 4, dt * 128:(dt + 1) * 128],
                            pt4.rearrange("p (a b) -> p a b", a=4))

                for sc in range(SC):
                    psums = [ppj.tile([128, 512], F32, tag=f"pj{d}", name=f"pj{d}") for d in range(6)]
                    for part in range(16):  # h pieces of 256
                        hps = []
                        for st4 in range(4):
                            hp = hp_p.tile([128, 256], F32, tag=f"hp{st4}")
                            nc.sync.dma_start(
                                hp, hid_d[sc * 512 + st4 * 128: sc * 512 + (st4 + 1) * 128,
                                          part * 256:(part + 1) * 256])
                            hps.append(hp)
                        for j in range(2):
                            ho = part * 2 + j
                            ht = ht_p.tile([128, 512], F32R, tag="ht")
                            pt4 = ptr.tile([128, 512], F32, tag="tp")
                            for st4 in range(4):
                                nc.tensor.transpose(
                                    pt4[:, st4 * 128:(st4 + 1) * 128],
                                    hps[st4][:, j * 128:(j + 1) * 128], ident)
                            nc.scalar.copy(ht, pt4)
                            for d in range(6):
                                nc.tensor.matmul(
                                    psums[d], wt[:, ho, d * 128:(d + 1) * 128], ht,
                                    start=(ho == 0), stop=(ho == HO - 1))
                    # RoPE (strips 0-4) / copy (strip 5 = V)
                    cos_c = cs_p.tile([128, 512], F32, tag="cosc")
                    sin_c = cs_p.tile([128, 512], F32, tag="sinc")
                    nc.sync.dma_start(cos_c, cos_d[:, sc * 512:(sc + 1) * 512])
                    nc.sync.dma_start(sin_c, sin_d[:, sc * 512:(sc + 1) * 512])
                    for d in range(6):
                        dst = strips[:, d, sc * 512:(sc + 1) * 512]
                        if d < 5:
                            t1 = rt_p.tile([128, 512], F32, tag="t1")
                            t2 = rt_p.tile([128, 512], F32, tag="t2")
                            nc.vector.tensor_mul(t1, psums[d], cos_c)
                            nc.vector.tensor_mul(t2[0:64], psums[d][64:128], sin_c[0:64])
                            nc.vector.tensor_mul(t2[64:128], psums[d][0:64], sin_c[64:128])
                            nc.vector.tensor_add(dst, t1, t2)
                        else:
                            nc.scalar.copy(dst, psums[d])

            # ---------------- Phase C: attention ----------------
            with tc.tile_pool(name="perm2", bufs=1) as perm2:
                vnat = perm2.tile([128, 16, 128], F32R, tag="vnat")
                attnT = perm2.tile([128, NQ, S], F32R, tag="attnT")
                ones_m = perm2.tile([128, 512], F32, tag="ones_m")
                nc.gpsimd.memset(ones_m, 1.0)
                masks = perm2.tile([128, 4, 512], F32, tag="masks")
                for j in range(4):
                    nc.gpsimd.affine_select(
                        out=masks[:, j, :], in_=ones_m, pattern=[[1, 512]],
                        compare_op=OP.is_ge, fill=0.0,
                        base=-128 * j, channel_multiplier=-1)
                with tc.tile_pool(name="pts", bufs=4) as pts_p, \
                     tc.tile_pool(name="rec", bufs=2) as rec_p, \
                     tc.tile_pool(name="ps_s", bufs=2, space="PSUM") as s_p, \
                     tc.tile_pool(name="ps_pv", bufs=2, space="PSUM") as pv_p, \
                     tc.tile_pool(name="ps_dn", bufs=2, space="PSUM") as dn_p, \
                     tc.tile_pool(name="ptrc", bufs=2, space="PSUM") as ptrc:
                    # V natural tiles from V^T strip
                    for g in range(4):
                        pt4 = ptrc.tile([128, 512], F32R, tag="tpc")
                        for i in range(4):
                            st = 4 * g + i
                            nc.tensor.transpose(
                                pt4[:, i * 128:(i + 1) * 128],
                                strips[:, 5, st * 128:(st + 1) * 128], identr)
                        nc.vector.tensor_copy(
                            vnat[:, 4 * g:4 * g + 4, :],
                            pt4.rearrange("p (a b) -> p a b", a=4))

                    for h in range(NQ):
                        for c in range(SC):
                            nkt = 4 * c + 4
                            pv = pv_p.tile([128, 512], F32, tag="pv")
                            den = dn_p.tile([128, 512], F32, tag="den")
                            for kt in range(nkt):
                                sp = s_p.tile([128, 512], F32, tag="s")
                                nc.tensor.matmul(
                                    sp, strips[:, 4, kt * 128:(kt + 1) * 128],
                                    strips[:, h, c * 512:(c + 1) * 512],
                                    start=True, stop=True)
                                ptile = pts_p.tile([128, 512], F32R, tag="pt")
                                nc.scalar.activation(ptile, sp, AF.Exp, scale=SCALE)
                                j = kt - 4 * c
                                if j >= 0:
                                    nc.vector.tensor_mul(ptile, ptile, masks[:, j, :])
                                nc.tensor.matmul(pv, vnat[:, kt, :], ptile,
                                                 start=(kt == 0), stop=(kt == nkt - 1))
                                nc.tensor.matmul(den, ones, ptile,
                                                 start=(kt == 0), stop=(kt == nkt - 1))
                            rec = rec_p.tile([128, 512], F32, tag="rec")
                            nc.vector.reciprocal(rec, den)
                            nc.vector.tensor_mul(
                                attnT[:, h, c * 512:(c + 1) * 512], pv, rec)

                # ---------------- Phase D: o_proj ----------------
                with tc.tile_pool(name="wotp", bufs=1) as wotp, \
                     tc.tile_pool(name="pos", bufs=4) as pos_p, \
                     tc.tile_pool(name="won", bufs=2) as won_p, \
                     tc.tile_pool(name="po", bufs=4, space="PSUM") as po_p, \
                     tc.tile_pool(name="ptrd", bufs=2, space="PSUM") as ptrd:
                    woT = wotp.tile([128, NQ, H], F32R, tag="woT")
                    for mt in range(32):
                        wn = won_p.tile([128, 512], F32, tag="won")
                        nc.sync.dma_start(wn, wo_d[mt * 128:(mt + 1) * 128, :])
                        pt4 = ptrd.tile([128, 512], F32, tag="tpd")
                        for at in range(4):
                            nc.tensor.transpose(
                                pt4[:, at * 128:(at + 1) * 128],
                                wn[:, at * 128:(at + 1) * 128], ident)
                        nc.scalar.copy(woT[:, :, mt * 128:(mt + 1) * 128],
                                       pt4.rearrange("p (a b) -> p a b", a=4))
                    for st in range(16):
                        for mc in range(8):
                            po = po_p.tile([128, 512], F32, tag="po")
                            for at in range(4):
                                nc.tensor.matmul(
                                    po, attnT[:, at, st * 128:(st + 1) * 128],
                                    woT[:, at, mc * 512:(mc + 1) * 512],
                                    start=(at == 0), stop=(at == 3))
                            pos = pos_p.tile([128, 512], F32, tag="pos")
                            nc.vector.tensor_copy(pos, po)
                            nc.sync.dma_start(
                                out_d[st * 128:(st + 1) * 128, mc * 512:(mc + 1) * 512],
                                pos)
    nc.compile()
    return nc


def kernel(hidden_states, position_ids, Wq, Wk, Wv, Wo, **extra):
    hidden_states = np.asarray(hidden_states, dtype=np.float32)
    position_ids = np.asarray(position_ids)
    Wq = np.asarray(Wq, dtype=np.float32)
    Wk = np.asarray(Wk, dtype=np.float32)
    Wv = np.asarray(Wv, dtype=np.float32)
    Wo = np.asarray(Wo, dtype=np.float32)
    B = hidden_states.shape[0]
    assert B == 1 and hidden_states.shape[1] == S and hidden_states.shape[2] == H

    if "nc" not in _CACHED:
        _CACHED["nc"] = _build_nc()
    nc = _CACHED["nc"]

    # RoPE tables in [d=128, s] layout; sin has sign folded for rotate_half
    pos = position_ids.reshape(-1).astype(np.float64)  # [S]
    invf = 1.0 / (10000.0 ** (np.arange(0, 128, 2, dtype=np.float64) / 128.0))  # [64]
    ang = invf[:, None] * pos[None, :]                 # [64, S]
    cos_t = np.concatenate([np.cos(ang), np.cos(ang)], axis=0).astype(np.float32)
    sin_t = np.concatenate([-np.sin(ang), np.sin(ang)], axis=0).astype(np.float32)

    hid = np.ascontiguousarray(hidden_states[0])
    in_maps = []
    for c in range(NCORES):
        wqkv = np.ascontiguousarray(np.concatenate([
            Wq[c * 512:(c + 1) * 512],
            Wk[c * 128:(c + 1) * 128],
            Wv[c * 128:(c + 1) * 128]], axis=0))
        wo_c = np.ascontiguousarray(Wo[:, c * 512:(c + 1) * 512])
        in_maps.append({"hidden": hid, "wqkv": wqkv, "wo": wo_c,
                        "cos": cos_t, "sin": sin_t})

    res = run_bass_kernel_spmd(nc, in_maps, core_ids=list(range(NCORES)))
    out = np.zeros((S, H), dtype=np.float32)
    for c in range(NCORES):
        out += res.results[c]["out"]
    return out.reshape(1, S, H)



# revision 2
# speedup vs baseline: 1.6354x; 1.6354x over previous
import sys
sys.path.insert(0, "/opt/trn_rl_repo")
import numpy as np
import ml_dtypes
import concourse.bass as bass
import concourse.mybir as mybir
import concourse.tile as tile
from concourse import bacc
from concourse.bass_utils import run_bass_kernel_spmd
from concourse.masks import make_identity

F32 = mybir.dt.float32
BF16 = mybir.dt.bfloat16
AF = mybir.ActivationFunctionType
OP = mybir.AluOpType

S = 2048          # sequence length
H = 4096          # hidden dim
DHEAD = 128       # head dim
NQ = 4            # q heads per core
NCORES = 8
SC = 4            # s-chunks of 512
HO = 32           # h k-tiles of 128
SCALE = 1.0 / np.sqrt(128.0)
BF = ml_dtypes.bfloat16

_CACHED = {}


def _build_nc():
    nc = bacc.Bacc(None, target_bir_lowering=False, debug=False)
    # all inputs pre-transposed / packed host-side, bf16
    hidT_d = nc.dram_tensor("hidT", [H, S], BF16, kind="ExternalInput")
    wqkvT_d = nc.dram_tensor("wqkvT", [H, 768], BF16, kind="ExternalInput")
    woT_d = nc.dram_tensor("woT", [512, H], BF16, kind="ExternalInput")
    cos_d = nc.dram_tensor("cos", [128, S], F32, kind="ExternalInput")
    sin_d = nc.dram_tensor("sin", [128, S], F32, kind="ExternalInput")
    out_d = nc.dram_tensor("out", [S, H], BF16, kind="ExternalOutput")

    with tile.TileContext(nc) as tc:
        with tc.tile_pool(name="perm", bufs=1) as perm:
            ident = perm.tile([128, 128], BF16, tag="ident")
            make_identity(nc, ident)
            ones = perm.tile([128, 128], BF16, tag="ones")
            nc.gpsimd.memset(ones, 1.0)
            ones_m = perm.tile([128, 512], BF16, tag="ones_m")
            nc.gpsimd.memset(ones_m, 1.0)
            masks = perm.tile([128, 4, 512], BF16, tag="masks")
            for j in range(4):
                nc.gpsimd.affine_select(
                    out=masks[:, j, :], in_=ones_m, pattern=[[1, 512]],
                    compare_op=OP.is_ge, fill=0.0,
                    base=-128 * j, channel_multiplier=-1)
            # qT/kT/vT strips, [d=128, strip, s]: strips 0-3 = Q heads, 4 = K, 5 = V
            strips = perm.tile([128, 6, S], BF16, tag="strips")
            wq_s = perm.tile([128, HO, 768], BF16, tag="wq_s")
            wo_s = perm.tile([128, NQ, H], BF16, tag="wo_s")
            attnT = perm.tile([128, NQ, S], BF16, tag="attnT")
            vnat = perm.tile([128, 16, 128], BF16, tag="vnat")

            for ho in range(HO):
                nc.sync.dma_start(wq_s[:, ho, :], wqkvT_d[ho * 128:(ho + 1) * 128, :])

            # ---------------- Phase B: projections + RoPE ----------------
            with tc.tile_pool(name="hp", bufs=4) as hp_p, \
                 tc.tile_pool(name="cs", bufs=2) as cs_p, \
                 tc.tile_pool(name="rt", bufs=4) as rt_p, \
                 tc.tile_pool(name="ppj", bufs=1, space="PSUM") as ppj:
                for sc in range(SC):
                    cos_c = cs_p.tile([128, 512], F32, tag="cosc")
                    sin_c = cs_p.tile([128, 512], F32, tag="sinc")
                    nc.sync.dma_start(cos_c, cos_d[:, sc * 512:(sc + 1) * 512])
                    nc.sync.dma_start(sin_c, sin_d[:, sc * 512:(sc + 1) * 512])
                    psums = [ppj.tile([128, 512], F32, tag=f"pj{d}", name=f"pj{d}")
                             for d in range(6)]
                    for ho in range(HO):
                        ht = hp_p.tile([128, 512], BF16, tag="ht")
                        nc.sync.dma_start(
                            ht, hidT_d[ho * 128:(ho + 1) * 128, sc * 512:(sc + 1) * 512])
                        for d in range(6):
                            nc.tensor.matmul(
                                psums[d], wq_s[:, ho, d * 128:(d + 1) * 128], ht,
                                start=(ho == 0), stop=(ho == HO - 1))
                    # RoPE (strips 0-4) / copy (strip 5 = V)
                    for d in range(6):
                        dst = strips[:, d, sc * 512:(sc + 1) * 512]
                        if d < 5:
                            t1 = rt_p.tile([128, 512], F32, tag="t1")
                            t2 = rt_p.tile([128, 512], F32, tag="t2")
                            nc.vector.tensor_mul(t1, psums[d], cos_c)
                            nc.vector.tensor_mul(t2[0:64], psums[d][64:128], sin_c[0:64])
                            nc.vector.tensor_mul(t2[64:128], psums[d][0:64], sin_c[64:128])
                            nc.vector.tensor_add(dst, t1, t2)
                        else:
                            nc.scalar.copy(dst, psums[d])

            # ---------------- Phase C: attention ----------------
            for at in range(NQ):
                nc.sync.dma_start(wo_s[:, at, :], woT_d[at * 128:(at + 1) * 128, :])
            with tc.tile_pool(name="pts", bufs=4) as pts_p, \
                 tc.tile_pool(name="rec", bufs=2) as rec_p, \
                 tc.tile_pool(name="ps_s", bufs=3, space="PSUM") as s_p, \
                 tc.tile_pool(name="ps_pv", bufs=2, space="PSUM") as pv_p, \
                 tc.tile_pool(name="ps_dn", bufs=2, space="PSUM") as dn_p, \
                 tc.tile_pool(name="ptrc", bufs=1, space="PSUM") as ptrc:
                # V natural tiles from V^T strip
                for g in range(4):
                    pt4 = ptrc.tile([128, 512], BF16, tag="tpc")
                    for i in range(4):
                        st = 4 * g + i
                        nc.tensor.transpose(
                            pt4[:, i * 128:(i + 1) * 128],
                            strips[:, 5, st * 128:(st + 1) * 128], ident)
                    nc.vector.tensor_copy(
                        vnat[:, 4 * g:4 * g + 4, :],
                        pt4.rearrange("p (a b) -> p a b", a=4))

                for h in range(NQ):
                    for c in range(SC):
                        nkt = 4 * c + 4
                        pv = pv_p.tile([128, 512], F32, tag="pv")
                        den = dn_p.tile([128, 512], F32, tag="den")
                        for kt in range(nkt):
                            sp = s_p.tile([128, 512], F32, tag="s")
                            nc.tensor.matmul(
                                sp, strips[:, 4, kt * 128:(kt + 1) * 128],
                                strips[:, h, c * 512:(c + 1) * 512],
                                start=True, stop=True)
                            ptile = pts_p.tile([128, 512], BF16, tag="pt")
                            nc.scalar.activation(ptile, sp, AF.Exp, scale=SCALE)
                            j = kt - 4 * c
                            if j >= 0:
                                nc.vector.tensor_mul(ptile, ptile, masks[:, j, :])
                            nc.tensor.matmul(pv, vnat[:, kt, :], ptile,
                                             start=(kt == 0), stop=(kt == nkt - 1))
                            nc.tensor.matmul(den, ones, ptile,
                                             start=(kt == 0), stop=(kt == nkt - 1))
                        rec = rec_p.tile([128, 512], F32, tag="rec")
                        nc.vector.reciprocal_approx_fast(rec, den)
                        nc.vector.tensor_mul(
                            attnT[:, h, c * 512:(c + 1) * 512], pv, rec)

            # ---------------- Phase D: o_proj ----------------
            with tc.tile_pool(name="pos", bufs=2) as pos_p, \
                 tc.tile_pool(name="po", bufs=4, space="PSUM") as po_p:
                for st in range(16):
                    pos = pos_p.tile([128, H], BF16, tag="pos")
                    for mc in range(8):
                        po = po_p.tile([128, 512], F32, tag="po")
                        for at in range(NQ):
                            nc.tensor.matmul(
                                po, attnT[:, at, st * 128:(st + 1) * 128],
                                wo_s[:, at, mc * 512:(mc + 1) * 512],
                                start=(at == 0), stop=(at == 3))
                        nc.scalar.copy(pos[:, mc * 512:(mc + 1) * 512], po)
                    nc.sync.dma_start(out_d[st * 128:(st + 1) * 128, :], pos)
    nc.compile()
    return nc


def _prepare(hidden_states, position_ids, Wq, Wk, Wv, Wo):
    hidden_states = np.asarray(hidden_states, dtype=np.float32)
    position_ids = np.asarray(position_ids)
    Wq = np.asarray(Wq, dtype=np.float32)
    Wk = np.asarray(Wk, dtype=np.float32)
    Wv = np.asarray(Wv, dtype=np.float32)
    Wo = np.asarray(Wo, dtype=np.float32)

    # RoPE tables in [d=128, s] layout; sin has sign folded for rotate_half
    pos = position_ids.reshape(-1).astype(np.float64)  # [S]
    invf = 1.0 / (10000.0 ** (np.arange(0, 128, 2, dtype=np.float64) / 128.0))
    ang = invf[:, None] * pos[None, :]                 # [64, S]
    cos_t = np.concatenate([np.cos(ang), np.cos(ang)], axis=0).astype(np.float32)
    sin_t = np.concatenate([-np.sin(ang), np.sin(ang)], axis=0).astype(np.float32)

    hidT = np.ascontiguousarray(hidden_states[0].T).astype(BF)   # [H, S]
    WoT = np.ascontiguousarray(Wo.T)                             # [H, H]
    in_maps = []
    for c in range(NCORES):
        wqkvT = np.ascontiguousarray(np.concatenate([
            Wq[c * 512:(c + 1) * 512],
            Wk[c * 128:(c + 1) * 128],
            Wv[c * 128:(c + 1) * 128]], axis=0).T).astype(BF)    # [H, 768]
        woT_c = np.ascontiguousarray(WoT[c * 512:(c + 1) * 512]).astype(BF)  # [512, H]
        in_maps.append({"hidT": hidT, "wqkvT": wqkvT, "woT": woT_c,
                        "cos": cos_t, "sin": sin_t})
    return in_maps


def kernel(hidden_states, position_ids, Wq, Wk, Wv, Wo, **extra):
    B = np.asarray(hidden_states).shape[0]
    assert B == 1

    if "nc" not in _CACHED:
        _CACHED["nc"] = _build_nc()
    nc = _CACHED["nc"]

    in_maps = _prepare(hidden_states, position_ids, Wq, Wk, Wv, Wo)
    res = run_bass_kernel_spmd(nc, in_maps, core_ids=list(range(NCORES)))
    out = np.zeros((S, H), dtype=np.float32)
    for c in range(NCORES):
        out += res.results[c]["out"].astype(np.float32)
    return out.reshape(1, S, H)


# revision 3
# speedup vs baseline: 1.7224x; 1.0532x over previous
import sys
sys.path.insert(0, "/opt/trn_rl_repo")
import numpy as np
import ml_dtypes
import concourse.bass as bass
import concourse.mybir as mybir
import concourse.tile as tile
from concourse import bacc
from concourse.bass_utils import run_bass_kernel_spmd
from concourse.masks import make_identity

F32 = mybir.dt.float32
BF16 = mybir.dt.bfloat16
AF = mybir.ActivationFunctionType
OP = mybir.AluOpType

S = 2048          # sequence length
H = 4096          # hidden dim
DHEAD = 128       # head dim
NQ = 4            # q heads per core
NCORES = 8
SC = 4            # s-chunks of 512
HO = 32           # h k-tiles of 128
SCALE = 1.0 / np.sqrt(128.0)
BF = ml_dtypes.bfloat16

_CACHED = {}


def _build_nc():
    nc = bacc.Bacc(None, target_bir_lowering=False, debug=False)
    # all inputs pre-transposed / packed host-side, bf16
    hidT_d = nc.dram_tensor("hidT", [H, S], BF16, kind="ExternalInput")
    wqkvT_d = nc.dram_tensor("wqkvT", [H, 768], BF16, kind="ExternalInput")
    woT_d = nc.dram_tensor("woT", [512, H], BF16, kind="ExternalInput")
    cos_d = nc.dram_tensor("cos", [128, S], F32, kind="ExternalInput")
    sin_d = nc.dram_tensor("sin", [128, S], F32, kind="ExternalInput")
    out_d = nc.dram_tensor("out", [S, H], BF16, kind="ExternalOutput")

    with tile.TileContext(nc) as tc:
        with tc.tile_pool(name="perm", bufs=1) as perm:
            ident = perm.tile([128, 128], BF16, tag="ident")
            make_identity(nc, ident)
            ones = perm.tile([128, 128], BF16, tag="ones")
            nc.gpsimd.memset(ones, 1.0)
            ones_m = perm.tile([128, 512], BF16, tag="ones_m")
            nc.gpsimd.memset(ones_m, 1.0)
            masks = perm.tile([128, 4, 512], BF16, tag="masks")
            for j in range(4):
                nc.gpsimd.affine_select(
                    out=masks[:, j, :], in_=ones_m, pattern=[[1, 512]],
                    compare_op=OP.is_ge, fill=0.0,
                    base=-128 * j, channel_multiplier=-1)
            # qT/kT/vT strips, [d=128, strip, s]: strips 0-3 = Q heads, 4 = K, 5 = V
            strips = perm.tile([128, 6, S], BF16, tag="strips")
            wq_s = perm.tile([128, HO, 768], BF16, tag="wq_s")
            wo_s = perm.tile([128, NQ, H], BF16, tag="wo_s")
            attnT = perm.tile([128, NQ, S], BF16, tag="attnT")
            vnat = perm.tile([128, 16, 128], BF16, tag="vnat")

            # ---------------- Phase B: projections + RoPE ----------------
            with tc.tile_pool(name="hp", bufs=12) as hp_p, \
                 tc.tile_pool(name="cs", bufs=4) as cs_p, \
                 tc.tile_pool(name="rt", bufs=4) as rt_p, \
                 tc.tile_pool(name="ppj", bufs=1, space="PSUM") as ppj:
                # prefetch all cos/sin chunks up front
                cs_tiles = []
                for sc in range(SC):
                    cos_c = cs_p.tile([128, 512], F32, tag="cosc")
                    sin_c = cs_p.tile([128, 512], F32, tag="sinc")
                    nc.sync.dma_start(cos_c, cos_d[:, sc * 512:(sc + 1) * 512])
                    nc.sync.dma_start(sin_c, sin_d[:, sc * 512:(sc + 1) * 512])
                    cs_tiles.append((cos_c, sin_c))
                for sc in range(SC):
                    cos_c, sin_c = cs_tiles[sc]
                    psums = [ppj.tile([128, 512], F32, tag=f"pj{d}", name=f"pj{d}")
                             for d in range(6)]
                    for ho in range(HO):
                        if sc == 0:
                            # interleave weight strip loads with the first
                            # chunk's hidden tiles so the PE starts immediately
                            nc.sync.dma_start(
                                wq_s[:, ho, :], wqkvT_d[ho * 128:(ho + 1) * 128, :])
                        ht = hp_p.tile([128, 512], BF16, tag="ht")
                        nc.sync.dma_start(
                            ht, hidT_d[ho * 128:(ho + 1) * 128, sc * 512:(sc + 1) * 512])
                        for d in range(6):
                            nc.tensor.matmul(
                                psums[d], wq_s[:, ho, d * 128:(d + 1) * 128], ht,
                                start=(ho == 0), stop=(ho == HO - 1))
                    if sc == 0:
                        # o_proj weights prefetch (DMA has headroom from here on)
                        for at in range(NQ):
                            nc.sync.dma_start(
                                wo_s[:, at, :], woT_d[at * 128:(at + 1) * 128, :])
                    # RoPE (strips 0-4) / copy (strip 5 = V)
                    for d in range(6):
                        dst = strips[:, d, sc * 512:(sc + 1) * 512]
                        if d < 5:
                            t1 = rt_p.tile([128, 512], F32, tag="t1")
                            t2 = rt_p.tile([128, 512], F32, tag="t2")
                            nc.vector.tensor_mul(t1, psums[d], cos_c)
                            nc.vector.tensor_mul(t2[0:64], psums[d][64:128], sin_c[0:64])
                            nc.vector.tensor_mul(t2[64:128], psums[d][0:64], sin_c[64:128])
                            nc.vector.tensor_add(dst, t1, t2)
                        else:
                            nc.scalar.copy(dst, psums[d])
                # V natural tiles from V^T strip (PE transposes, trivial cost)
                for g in range(4):
                    pt4 = ppj.tile([128, 512], BF16, tag="vtr")
                    for i in range(4):
                        st = 4 * g + i
                        nc.tensor.transpose(
                            pt4[:, i * 128:(i + 1) * 128],
                            strips[:, 5, st * 128:(st + 1) * 128], ident)
                    nc.vector.tensor_copy(
                        vnat[:, 4 * g:4 * g + 4, :],
                        pt4.rearrange("p (a b) -> p a b", a=4))

            # ---------------- Phase C: attention ----------------
            with tc.tile_pool(name="pts", bufs=4) as pts_p, \
                 tc.tile_pool(name="rec", bufs=2) as rec_p, \
                 tc.tile_pool(name="ps_s", bufs=2, space="PSUM") as s_p, \
                 tc.tile_pool(name="ps_pv", bufs=2, space="PSUM") as pv_p, \
                 tc.tile_pool(name="ps_dn", bufs=2, space="PSUM") as dn_p:
                for h in range(NQ):
                    for c in range(SC):
                        nkt = 4 * c + 4
                        pv = pv_p.tile([128, 512], F32, tag="pv")
                        den = dn_p.tile([128, 512], F32, tag="den")
                        for p in range(nkt // 2):
                            sp = s_p.tile([128, 2, 512], F32, tag="s")
                            for j in range(2):
                                kt = 2 * p + j
                                nc.tensor.matmul(
                                    sp[:, j, :],
                                    strips[:, 4, kt * 128:(kt + 1) * 128],
                                    strips[:, h, c * 512:(c + 1) * 512],
                                    start=True, stop=True)
                            ptile = pts_p.tile([128, 2, 512], BF16, tag="pt")
                            nc.scalar.activation(ptile, sp, AF.Exp, scale=SCALE)
                            jm = 2 * p - 4 * c
                            if jm >= 0:
                                nc.vector.tensor_mul(
                                    ptile, ptile, masks[:, jm:jm + 2, :])
                            for j in range(2):
                                kt = 2 * p + j
                                nc.tensor.matmul(pv, vnat[:, kt, :], ptile[:, j, :],
                                                 start=(kt == 0), stop=(kt == nkt - 1))
                                nc.tensor.matmul(den, ones, ptile[:, j, :],
                                                 start=(kt == 0), stop=(kt == nkt - 1))
                        rec = rec_p.tile([128, 512], F32, tag="rec")
                        nc.vector.reciprocal_approx_fast(rec, den)
                        nc.vector.tensor_mul(
                            attnT[:, h, c * 512:(c + 1) * 512], pv, rec)

            # ---------------- Phase D: o_proj ----------------
            with tc.tile_pool(name="pos", bufs=2) as pos_p, \
                 tc.tile_pool(name="po", bufs=4, space="PSUM") as po_p:
                for st in range(16):
                    pos = pos_p.tile([128, H], BF16, tag="pos")
                    for mc in range(8):
                        po = po_p.tile([128, 512], F32, tag="po")
                        for at in range(NQ):
                            nc.tensor.matmul(
                                po, attnT[:, at, st * 128:(st + 1) * 128],
                                wo_s[:, at, mc * 512:(mc + 1) * 512],
                                start=(at == 0), stop=(at == 3))
                        nc.scalar.copy(pos[:, mc * 512:(mc + 1) * 512], po)
                    nc.sync.dma_start(out_d[st * 128:(st + 1) * 128, :], pos)
    nc.compile()
    return nc


def _prepare(hidden_states, position_ids, Wq, Wk, Wv, Wo):
    hidden_states = np.asarray(hidden_states, dtype=np.float32)
    position_ids = np.asarray(position_ids)
    Wq = np.asarray(Wq, dtype=np.float32)
    Wk = np.asarray(Wk, dtype=np.float32)
    Wv = np.asarray(Wv, dtype=np.float32)
    Wo = np.asarray(Wo, dtype=np.float32)

    # RoPE tables in [d=128, s] layout; sin has sign folded for rotate_half
    pos = position_ids.reshape(-1).astype(np.float64)  # [S]
    invf = 1.0 / (10000.0 ** (np.arange(0, 128, 2, dtype=np.float64) / 128.0))
    ang = invf[:, None] * pos[None, :]                 # [64, S]
    cos_t = np.concatenate([np.cos(ang), np.cos(ang)], axis=0).astype(np.float32)
    sin_t = np.concatenate([-np.sin(ang), np.sin(ang)], axis=0).astype(np.float32)

    hidT = np.ascontiguousarray(hidden_states[0].T).astype(BF)   # [H, S]
    WoT = np.ascontiguousarray(Wo.T)                             # [H, H]
    in_maps = []
    for c in range(NCORES):
        wqkvT = np.ascontiguousarray(np.concatenate([
            Wq[c * 512:(c + 1) * 512],
            Wk[c * 128:(c + 1) * 128],
            Wv[c * 128:(c + 1) * 128]], axis=0).T).astype(BF)    # [H, 768]
        woT_c = np.ascontiguousarray(WoT[c * 512:(c + 1) * 512]).astype(BF)  # [512, H]
        in_maps.append({"hidT": hidT, "wqkvT": wqkvT, "woT": woT_c,
                        "cos": cos_t, "sin": sin_t})
    return in_maps


def kernel(hidden_states, position_ids, Wq, Wk, Wv, Wo, **extra):
    B = np.asarray(hidden_states).shape[0]
    assert B == 1

    if "nc" not in _CACHED:
        _CACHED["nc"] = _build_nc()
    nc = _CACHED["nc"]

    in_maps = _prepare(hidden_states, position_ids, Wq, Wk, Wv, Wo)
    res = run_bass_kernel_spmd(nc, in_maps, core_ids=list(range(NCORES)))
    out = np.zeros((S, H), dtype=np.float32)
    for c in range(NCORES):
        out += res.results[c]["out"].astype(np.float32)
    return out.reshape(1, S, H)


# revision 7
# speedup vs baseline: 1.8673x; 1.0841x over previous
import sys
sys.path.insert(0, "/opt/trn_rl_repo")
import numpy as np
import ml_dtypes
import concourse.bass as bass
import concourse.mybir as mybir
import concourse.tile as tile
from concourse import bacc
from concourse.bass_utils import run_bass_kernel_spmd
from concourse.masks import make_identity

F32 = mybir.dt.float32
BF16 = mybir.dt.bfloat16
AF = mybir.ActivationFunctionType
OP = mybir.AluOpType

S = 2048          # sequence length
H = 4096          # hidden dim
DHEAD = 128       # head dim
NQ = 4            # q heads per core
NCORES = 8
SC = 4            # s-chunks of 512
HO = 32           # h k-tiles of 128
SCALE = 1.0 / np.sqrt(128.0)
BF = ml_dtypes.bfloat16

_CACHED = {}


def _build_nc():
    nc = bacc.Bacc(None, target_bir_lowering=False, debug=False)
    # all inputs pre-swizzled host-side into SBUF partition layout, bf16
    hid_d = nc.dram_tensor("hidp", [128, SC * HO * 512], BF16, kind="ExternalInput")
    wq_d = nc.dram_tensor("wqp", [128, 6 * HO * 128], BF16, kind="ExternalInput")
    wo_d = nc.dram_tensor("wop", [128, NQ * H], BF16, kind="ExternalInput")
    cos_d = nc.dram_tensor("cos", [128, S], BF16, kind="ExternalInput")
    sin_d = nc.dram_tensor("sin", [128, S], BF16, kind="ExternalInput")
    out_d = nc.dram_tensor("out", [S, H], BF16, kind="ExternalOutput")

    with tile.TileContext(nc) as tc:
        with tc.tile_pool(name="perm", bufs=1) as perm:
            ident = perm.tile([128, 128], BF16, tag="ident")
            make_identity(nc, ident)
            ones = perm.tile([128, 128], BF16, tag="ones")
            nc.gpsimd.memset(ones, 1.0)
            # qT/kT/vT strips, [d=128, strip, s]: strips 0-3 = Q heads, 4 = K, 5 = V
            strips = perm.tile([128, 6, S], BF16, tag="strips")
            wq_s = perm.tile([128, 6, HO * 128], BF16, tag="wq_s")
            wo_s = perm.tile([128, NQ, H], BF16, tag="wo_s")
            attnT = perm.tile([128, NQ, S], BF16, tag="attnT")
            vnat = perm.tile([128, 16, 128], BF16, tag="vnat")
            cos_s = perm.tile([128, S], BF16, tag="cos_s")
            sin_s = perm.tile([128, S], BF16, tag="sin_s")

            # ---------------- Phase B: projections + RoPE ----------------
            # d-outer accumulation: RoPE of group d overlaps matmuls of d+1
            with tc.tile_pool(name="hc", bufs=2) as hc_p, \
                 tc.tile_pool(name="rt", bufs=2) as rt_p, \
                 tc.tile_pool(name="ppj", bufs=1, space="PSUM") as ppj:
                nc.sync.dma_start(wq_s[:, 0, :], wq_d[:, 0:HO * 128])
                htcs = []
                for sc in range(SC):
                    htc = hc_p.tile([128, HO, 512], BF16, tag="htc")
                    base = sc * HO * 512
                    if sc == 0:
                        for sub in range(4):
                            nc.sync.dma_start(
                                htc[:, sub * 8:(sub + 1) * 8, :],
                                hid_d[:, base + sub * 4096: base + (sub + 1) * 4096])
                    else:
                        nc.sync.dma_start(htc, hid_d[:, base:base + HO * 512])
                    htcs.append(htc)
                    if sc == 0:
                        nc.sync.dma_start(cos_s, cos_d[:, 0:S])
                        nc.sync.dma_start(sin_s, sin_d[:, 0:S])
                        for d in range(1, 6):
                            nc.sync.dma_start(
                                wq_s[:, d, :], wq_d[:, d * HO * 128:(d + 1) * HO * 128])
                for sc in range(SC):
                    htc = htcs[sc]
                    for d in range(6):
                        psum = ppj.tile([128, 512], F32, tag=f"pj{d}", name=f"pj{d}")
                        for ho in range(HO):
                            nc.tensor.matmul(
                                psum, wq_s[:, d, ho * 128:(ho + 1) * 128],
                                htc[:, ho, :],
                                start=(ho == 0), stop=(ho == HO - 1))
                        # RoPE (strips 0-4) / copy (strip 5 = V)
                        dst = strips[:, d, sc * 512:(sc + 1) * 512]
                        cos_c = cos_s[:, sc * 512:(sc + 1) * 512]
                        sin_c = sin_s[:, sc * 512:(sc + 1) * 512]
                        if d < 5:
                            t1 = rt_p.tile([128, 512], F32, tag="t1")
                            t2 = rt_p.tile([128, 512], F32, tag="t2")
                            nc.vector.tensor_mul(t1, psum, cos_c)
                            nc.vector.tensor_mul(t2[0:64], psum[64:128], sin_c[0:64])
                            nc.vector.tensor_mul(t2[64:128], psum[0:64], sin_c[64:128])
                            nc.vector.tensor_add(dst, t1, t2)
                        else:
                            nc.scalar.copy(dst, psum)
                    # V natural tiles for this chunk (PE transposes, trivial)
                    pt4 = ppj.tile([128, 512], BF16, tag="vtr")
                    for i in range(4):
                        st = 4 * sc + i
                        nc.tensor.transpose(
                            pt4[:, i * 128:(i + 1) * 128],
                            strips[:, 5, st * 128:(st + 1) * 128], ident)
                    nc.vector.tensor_copy(
                        vnat[:, 4 * sc:4 * sc + 4, :],
                        pt4.rearrange("p (a b) -> p a b", a=4))

            # ---------------- Phase C: attention (head-pair interleaved) --------
            for at in range(NQ):
                nc.sync.dma_start(wo_s[:, at, :], wo_d[:, at * H:(at + 1) * H])
            with tc.tile_pool(name="pmask", bufs=1) as pmask, \
                 tc.tile_pool(name="pts", bufs=4) as pts_p, \
                 tc.tile_pool(name="rec", bufs=2) as rec_p, \
                 tc.tile_pool(name="ps_s", bufs=2, space="PSUM") as s_p, \
                 tc.tile_pool(name="ps_pv", bufs=1, space="PSUM") as pv_p, \
                 tc.tile_pool(name="ps_dn", bufs=1, space="PSUM") as dn_p:
                ones_m = pmask.tile([128, 512], BF16, tag="ones_m")
                nc.gpsimd.memset(ones_m, 1.0)
                masks = pmask.tile([128, 4, 512], BF16, tag="masks")
                for j in range(4):
                    nc.gpsimd.affine_select(
                        out=masks[:, j, :], in_=ones_m, pattern=[[1, 512]],
                        compare_op=OP.is_ge, fill=0.0,
                        base=-128 * j, channel_multiplier=-1)
                for hp in range(2):
                    heads = (2 * hp, 2 * hp + 1)
                    for c in range(SC):
                        nkt = 4 * c + 4
                        pvs = [pv_p.tile([128, 512], F32, tag=f"pv{i}", name=f"pv{i}")
                               for i in range(2)]
                        dens = [dn_p.tile([128, 512], F32, tag=f"dn{i}", name=f"dn{i}")
                                for i in range(2)]
                        for p in range(nkt // 2):
                            ptiles = []
                            for i, h in enumerate(heads):
                                sp = s_p.tile([128, 2, 512], F32, tag="s")
                                for j in range(2):
                                    kt = 2 * p + j
                                    nc.tensor.matmul(
                                        sp[:, j, :],
                                        strips[:, 4, kt * 128:(kt + 1) * 128],
                                        strips[:, h, c * 512:(c + 1) * 512],
                                        start=True, stop=True)
                                ptile = pts_p.tile([128, 2, 512], BF16, tag="pt")
                                nc.scalar.activation(ptile, sp, AF.Exp, scale=SCALE)
                                jm = 2 * p - 4 * c
                                if jm >= 0:
                                    nc.vector.tensor_mul(
                                        ptile, ptile, masks[:, jm:jm + 2, :])
                                ptiles.append(ptile)
                            for i in range(2):
                                for j in range(2):
                                    kt = 2 * p + j
                                    nc.tensor.matmul(
                                        pvs[i], vnat[:, kt, :], ptiles[i][:, j, :],
                                        start=(kt == 0), stop=(kt == nkt - 1))
                                    nc.tensor.matmul(
                                        dens[i], ones, ptiles[i][:, j, :],
                                        start=(kt == 0), stop=(kt == nkt - 1))
                        for i, h in enumerate(heads):
                            rec = rec_p.tile([128, 512], F32, tag="rec")
                            nc.vector.reciprocal_approx_fast(rec, dens[i])
                            nc.vector.tensor_mul(
                                attnT[:, h, c * 512:(c + 1) * 512], pvs[i], rec)

            # ---------------- Phase D: o_proj ----------------
            with tc.tile_pool(name="pos", bufs=2) as pos_p, \
                 tc.tile_pool(name="po", bufs=4, space="PSUM") as po_p:
                for st in range(16):
                    pos = pos_p.tile([128, H], BF16, tag="pos")
                    for mc in range(8):
                        po = po_p.tile([128, 512], F32, tag="po")
                        for at in range(NQ):
                            nc.tensor.matmul(
                                po, attnT[:, at, st * 128:(st + 1) * 128],
                                wo_s[:, at, mc * 512:(mc + 1) * 512],
                                start=(at == 0), stop=(at == 3))
                        nc.scalar.copy(pos[:, mc * 512:(mc + 1) * 512], po)
                    nc.sync.dma_start(out_d[st * 128:(st + 1) * 128, :], pos)
    nc.compile()
    return nc


def _prepare(hidden_states, position_ids, Wq, Wk, Wv, Wo):
    hidden_states = np.asarray(hidden_states, dtype=np.float32)
    position_ids = np.asarray(position_ids)
    Wq = np.asarray(Wq, dtype=np.float32)
    Wk = np.asarray(Wk, dtype=np.float32)
    Wv = np.asarray(Wv, dtype=np.float32)
    Wo = np.asarray(Wo, dtype=np.float32)

    # RoPE tables in [d=128, s] layout; sin has sign folded for rotate_half
    pos = position_ids.reshape(-1).astype(np.float64)  # [S]
    invf = 1.0 / (10000.0 ** (np.arange(0, 128, 2, dtype=np.float64) / 128.0))
    ang = invf[:, None] * pos[None, :]                 # [64, S]
    cos_t = np.ascontiguousarray(np.concatenate([np.cos(ang), np.cos(ang)], axis=0)).astype(BF)
    sin_t = np.ascontiguousarray(np.concatenate([-np.sin(ang), np.sin(ang)], axis=0)).astype(BF)

    # hidden pack: [p, (sc*HO + ho)*512 + j] = hidden[sc*512+j, ho*128+p]
    hidT = hidden_states[0].T.astype(BF)                       # [H, S]
    hid_pack = np.ascontiguousarray(
        hidT.reshape(HO, 128, SC, 512).transpose(1, 2, 0, 3).reshape(128, SC * HO * 512))

    WoT = np.ascontiguousarray(Wo.T)                           # [H, H]
    in_maps = []
    for c in range(NCORES):
        wqkvT = np.concatenate([
            Wq[c * 512:(c + 1) * 512],
            Wk[c * 128:(c + 1) * 128],
            Wv[c * 128:(c + 1) * 128]], axis=0).T.astype(BF)   # [H, 768]
        # wq pack: [p, (d*HO + ho)*128 + j] = wqkvT[ho*128+p, d*128+j]
        wq_pack = np.ascontiguousarray(
            wqkvT.reshape(HO, 128, 6, 128).transpose(1, 2, 0, 3).reshape(128, 6 * HO * 128))
        # wo pack: [p, at*H + col] = WoT[c*512 + at*128 + p, col]
        woT_c = WoT[c * 512:(c + 1) * 512].astype(BF)          # [512, H]
        wo_pack = np.ascontiguousarray(
            woT_c.reshape(NQ, 128, H).transpose(1, 0, 2).reshape(128, NQ * H))
        in_maps.append({"hidp": hid_pack, "wqp": wq_pack, "wop": wo_pack,
                        "cos": cos_t, "sin": sin_t})
    return in_maps


def kernel(hidden_states, position_ids, Wq, Wk, Wv, Wo, **extra):
    B = np.asarray(hidden_states).shape[0]
    assert B == 1

    if "nc" not in _CACHED:
        _CACHED["nc"] = _build_nc()
    nc = _CACHED["nc"]

    in_maps = _prepare(hidden_states, position_ids, Wq, Wk, Wv, Wo)
    res = run_bass_kernel_spmd(nc, in_maps, core_ids=list(range(NCORES)))
    out = np.zeros((S, H), dtype=np.float32)
    for c in range(NCORES):
        out += res.results[c]["out"].astype(np.float32)
    return out.reshape(1, S, H)
